# revision 27
# baseline (speedup 1.0000x reference)
"""Trainium2 Bass kernel for the PointNet-style GNN (nn_PointNet_36137854829226).

Self-contained: takes FULL inputs, shards internally across 8 NeuronCores,
returns the FULL [64, 512] output.

Strategy (per core, SPMD over 8 cores):
  - Edges sorted by dst, sharded by dst node range (1250 nodes/core).
  - Per-dst edges padded into groups of K1=8 slots (pad = duplicate edge).
  - Conv layer math in transposed orientation M^T = [hid, edge-slots]:
      layer1:  U^T = relu(W1a_aug^T @ Xe^T)           (K=8 contraction)
      layer2:  M^T = W1b^T @ U^T  (8x8 chunked, float32r)
    Level-1 segment max = DVE tensor_reduce(max) over a [128, G, 8] PSUM
    view; results are PE-transposed on the fly and written as row-major
    G-table rows.  Level-2 = K2 indirect-DMA gathers + DVE max tree.
  - relu(max + b) with empty segments handled by a -bias zero-row.
  - The G-tables, H2 table, and the hp AllGather are split in two halves at
    the node-640 boundary (group-padding keeps tiles aligned) so the node
    phases and collectives overlap the tail of the edge phases.
  - conv2 gathers hp rows (hp = h @ w2a[:1024] + b2a, AllGathered) by edge
    src via indirect DMA, adds rel @ w2a[1024:], relu, PE-transposes.
  - Pooling: same two-level max over nodes grouped by graph, then
    AllReduce(max); final pooled @ wo + bo on every core.
"""
import os
import numpy as np

from concourse import bass, bacc, tile, mybir
from concourse.bass_utils import run_bass_kernel_spmd
from concourse.masks import make_identity

dt = mybir.dt

N = 10000
E = 160000
HID = 1024
OUT = 512
NG = 64
NC = 8
CHA = 5 * 128            # G-table/H2 split boundary (local node rows)
CHB0 = 5 * 128           # allgather chunk boundaries (local node rows)
CHB1 = 8 * 128
K1 = 4                   # edge slots per group
K1P = 8                  # pool slots per group
TILE = 512               # edge slots per device tile
GPT = TILE // K1         # groups per tile
PTILE = 128              # pool slots per tile
GP_TILE = PTILE // K1P   # pool groups per tile (16)

F32R = dt.float32r

LAST_RESULTS = None      # test harness reads exec_time_ns from here
_CACHE = {}


# ----------------------------------------------------------------------------
# host preprocessing
# ----------------------------------------------------------------------------

def _ceil(a, b):
    return -(-a // b)


def _groups_for_range(counts, offs, lo, hi):
    """Group the sorted-by-dst edges of nodes [lo, hi) into K1-slot groups.
    Returns (slot_edge list, goff array[hi-lo+1])."""
    slot_edge = []
    goff = np.zeros(hi - lo + 1, np.int64)
    for n in range(lo, hi):
        deg = int(counts[n])
        gn = _ceil(deg, K1) if deg > 0 else 0
        goff[n - lo + 1] = goff[n - lo] + gn
        o = offs[n]
        for g in range(gn):
            for k in range(K1):
                slot_edge.append(o + min(g * K1 + k, deg - 1))
    return slot_edge, goff


def _balanced_cuts(counts):
    """17 node boundaries at group-count 16-quantiles: core c covers
    [cuts[2c], cuts[2c+2]) with its A/B split at cuts[2c+1]."""
    gcnt = -(-counts // K1)
    cum = np.concatenate([[0], np.cumsum(gcnt)])
    total = cum[-1]
    cuts = [0]
    for q in range(1, 2 * NC):
        cuts.append(int(np.searchsorted(cum, total * q / (2 * NC))))
    cuts.append(N)
    return cuts


def _build_edge_groups(src, dst):
    order = np.argsort(dst, kind="stable")
    s_dst = dst[order]
    counts = np.bincount(s_dst, minlength=N)
    offs = np.concatenate([[0], np.cumsum(counts)])
    cuts = _balanced_cuts(counts)

    halves = []   # per core: (seA, goffA, seB, goffB)
    gA_max, gB_max, k2_all = 0, 0, 1
    nA_max, nB_max = 0, 0
    for c in range(NC):
        lo, mid, hi = cuts[2 * c], cuts[2 * c + 1], cuts[2 * c + 2]
        seA, goffA = _groups_for_range(counts, offs, lo, mid)
        seB, goffB = _groups_for_range(counts, offs, mid, hi)
        nA_max = max(nA_max, mid - lo)
        nB_max = max(nB_max, hi - mid)
        gA_max = max(gA_max, int(goffA[-1]))
        gB_max = max(gB_max, int(goffB[-1]))
        for goff in (goffA, goffB):
            gn = np.diff(goff)
            if len(gn) and gn.max() > 0:
                k2_all = max(k2_all, int(gn.max()))
        halves.append((seA, goffA, seB, goffB))

    CHAD = _ceil(nA_max, 128) * 128
    NPCX = CHAD + _ceil(nB_max, 128) * 128
    GMID = _ceil(gA_max, GPT) * GPT
    GB = _ceil(gB_max, GPT) * GPT
    G = GMID + GB
    S = G * K1
    K2 = k2_all

    out = []
    for c in range(NC):
        seA, goffA, seB, goffB = halves[c]
        se = np.concatenate([
            np.array(seA, np.int64),
            np.zeros(GMID * K1 - len(seA), np.int64),
            np.array(seB, np.int64),
            np.zeros(GB * K1 - len(seB), np.int64),
        ])
        idx2 = np.empty((NPCX, K2), np.int32)
        idx2[:CHAD] = GMID                      # zrow of table A
        idx2[CHAD:] = GB                        # zrow of table B
        gnA, gnB = np.diff(goffA), np.diff(goffB)
        for n in range(len(gnA)):
            if gnA[n] > 0:
                for k in range(K2):
                    idx2[n, k] = goffA[n] + min(k, gnA[n] - 1)
        for n in range(len(gnB)):
            if gnB[n] > 0:
                for k in range(K2):
                    idx2[CHAD + n, k] = goffB[n] + min(k, gnB[n] - 1)
        out.append((se, idx2))
    return out, S, G, GMID, K2, order, cuts, CHAD, NPCX


def _pool_groups_for_range(b, lo, hi):
    """Pool groups for local nodes [lo, hi); returns (slot_node list,
    list of (graph, ngroups))."""
    slot_node = []
    gl = []
    for g in range(NG):
        nodes = lo + np.nonzero(b[lo:hi] == g)[0]
        if len(nodes) == 0:
            continue
        gn = _ceil(len(nodes), K1P)
        gl.append((g, gn))
        for j in range(gn):
            for k in range(K1P):
                slot_node.append(nodes[min(j * K1P + k, len(nodes) - 1)])
    return slot_node, gl


def _build_pool_groups(batch, cuts):
    per_core = []
    pA_max, pB_max = 0, 0
    for c in range(NC):
        b = batch[cuts[2 * c]:cuts[2 * c + 2]]
        nA = cuts[2 * c + 1] - cuts[2 * c]
        snA, glA = _pool_groups_for_range(b, 0, nA)
        snB, glB = _pool_groups_for_range(b, nA, len(b))
        snB = [x - nA for x in snB]
        pA_max = max(pA_max, sum(g for _, g in glA))
        pB_max = max(pB_max, sum(g for _, g in glB))
        per_core.append((snA, glA, snB, glB))

    PMID = _ceil(pA_max, GP_TILE) * GP_TILE
    PB = _ceil(pB_max, GP_TILE) * GP_TILE
    G3 = PMID + PB
    S3 = G3 * K1P

    out = []
    k3_all = 1
    pc2 = []
    for c in range(NC):
        snA, glA, snB, glB = per_core[c]
        # A-half slots reference H2A rows (node id), B-half H2B rows (-CHA)
        sn = np.concatenate([
            np.array(snA, np.int64),
            np.zeros(PMID * K1P - len(snA), np.int64),
            np.array(snB, np.int64),
            np.zeros(PB * K1P - len(snB), np.int64),
        ])
        # tile-internal layout p = k*16 + g
        sn_t = np.empty_like(sn)
        for t in range(S3 // PTILE):
            blk = sn[t * PTILE:(t + 1) * PTILE].reshape(GP_TILE, K1P)
            sn_t[t * PTILE:(t + 1) * PTILE] = blk.T.reshape(-1)
        # level-2 per graph over global G3 rows (A rows, then B at PMID)
        gmap = {}
        off = 0
        for g, gn in glA:
            gmap.setdefault(g, []).extend(range(off, off + gn))
            off += gn
        off = PMID
        for g, gn in glB:
            gmap.setdefault(g, []).extend(range(off, off + gn))
            off += gn
        k3_all = max(k3_all, max((len(v) for v in gmap.values()), default=1))
        pc2.append((sn_t, gmap))
    K3 = k3_all
    for c in range(NC):
        sn_t, gmap = pc2[c]
        idx3 = np.full((NG, K3), G3, np.int32)       # zrow of G3 table
        for g, rows in gmap.items():
            for k in range(K3):
                idx3[g, k] = rows[min(k, len(rows) - 1)]
        out.append((sn_t, idx3))
    return out, S3, G3, PMID, K3


def _preprocess(inputs):
    x = np.ascontiguousarray(np.asarray(inputs["x"], np.float32))
    ei = np.asarray(inputs["edge_index"])
    batch = np.asarray(inputs["batch"])
    src, dst = ei[0].astype(np.int64), ei[1].astype(np.int64)

    cores, S, G, GMID, K2, order, cuts, CHAD, NPCX = _build_edge_groups(src, dst)
    pools, S3, G3, PMID, K3 = _build_pool_groups(batch, cuts)
    s_src, s_dst = src[order], dst[order]

    w1a = np.asarray(inputs["w1a"], np.float32)
    b1a = np.asarray(inputs["b1a"], np.float32)
    w1b = np.asarray(inputs["w1b"], np.float32)
    b1b = np.asarray(inputs["b1b"], np.float32)
    w2a = np.asarray(inputs["w2a"], np.float32)
    b2a = np.asarray(inputs["b2a"], np.float32)
    w2b = np.asarray(inputs["w2b"], np.float32)
    b2b = np.asarray(inputs["b2b"], np.float32)
    wo = np.asarray(inputs["wo"], np.float32)
    bo = np.asarray(inputs["bo"], np.float32)

    w1a_aug = np.zeros((8, HID), np.float32)
    w1a_aug[0:6] = w1a
    w1a_aug[6] = b1a
    w2ar = np.zeros((4, HID), np.float32)
    w2ar[0:3] = w2a[HID:HID + 3]

    common = dict(
        w1a=np.ascontiguousarray(w1a_aug),
        w1b=np.ascontiguousarray(w1b),
        w2an=np.ascontiguousarray(w2a[:HID]),
        w2ar=np.ascontiguousarray(w2ar),
        w2b=np.ascontiguousarray(w2b),
        wo=np.ascontiguousarray(wo),
        b1b_bc=np.ascontiguousarray(np.broadcast_to(b1b, (128, HID))),
        b2a_bc=np.ascontiguousarray(np.broadcast_to(b2a, (128, HID))),
        b2b_bc=np.ascontiguousarray(np.broadcast_to(b2b, (128, HID))),
        bo_bc=np.ascontiguousarray(np.broadcast_to(bo, (64, OUT))),
        zr1=np.ascontiguousarray(-b1b[None, :]),
        zr2=np.ascontiguousarray(-b2b[None, :]),
        zr3=np.zeros((1, HID), np.float32),
    )

    in_maps = []
    for c in range(NC):
        se, idx2 = cores[c]
        sn_t, idx3 = pools[c]
        gsrc = s_src[se]
        gdst = s_dst[se]
        rel = (x[gsrc] - x[gdst]).T                       # [3, S]
        xeT8 = np.zeros((8, S), np.float32)
        xeT8[0:3] = x[gsrc].T
        xeT8[3:6] = rel
        xeT8[6] = 1.0
        relT4 = np.zeros((4, S), np.float32)
        relT4[0:3] = rel
        # hp_full row layout after 3-chunk AllGather; chunk boundaries at
        # device-local rows CHAD and CHB1D, each (core, local-row) ordered
        CHB1D = NPCX - 256
        lob = np.array([cuts[2 * cc] for cc in range(NC)] + [N])
        c_of = np.searchsorted(lob, gsrc, side="right") - 1
        midb = np.array([cuts[2 * cc + 1] for cc in range(NC)])
        j_raw = gsrc - lob[c_of]
        nA_c = midb[c_of] - lob[c_of]
        # device-local row: A-nodes at [0, nA_c), B-nodes at CHAD+
        j = np.where(j_raw < nA_c, j_raw, CHAD + (j_raw - nA_c))
        srcg_ag = np.where(
            j < CHAD, c_of * CHAD + j,
            np.where(j < CHB1D,
                     NC * CHAD + c_of * (CHB1D - CHAD) + (j - CHAD),
                     NC * CHB1D + c_of * (NPCX - CHB1D) + (j - CHB1D)))
        m = dict(common)
        m.update(
            xeT8=np.ascontiguousarray(xeT8),
            relT4=np.ascontiguousarray(relT4),
            srcg=np.ascontiguousarray(srcg_ag.astype(np.int32)),
            idx2=np.ascontiguousarray(idx2),
            pslot=np.ascontiguousarray(sn_t.astype(np.int32)),
            idx3=np.ascontiguousarray(idx3),
        )
        in_maps.append(m)
    return in_maps, (S, G, GMID, K2, S3, G3, PMID, K3, NPCX, CHAD)


# ----------------------------------------------------------------------------
# device program
# ----------------------------------------------------------------------------

def _build_program(S, G, GMID, K2, S3, G3, PMID, K3, NPCX, CHAD):
    NPCP = NPCX
    NT = NPCX // 128
    NTOT = NC * NPCX
    CHB0D = CHAD
    CHB1D = NPCX - 256
    nc = bacc.Bacc("TRN2", target_bir_lowering=False, debug=False,
                   num_devices=NC)
    f32 = dt.float32
    i32 = dt.int32

    def din(name, shape, dtype=f32):
        return nc.dram_tensor(name, shape, dtype, kind="ExternalInput")

    xeT8 = din("xeT8", [8, S])
    relT4 = din("relT4", [4, S])
    srcg = din("srcg", [S], i32)
    idx2 = din("idx2", [NPCP, K2], i32)
    pslot = din("pslot", [S3], i32)
    idx3 = din("idx3", [NG, K3], i32)
    w1a = din("w1a", [8, HID])
    w1b = din("w1b", [HID, HID])
    w2an = din("w2an", [HID, HID])
    w2ar = din("w2ar", [4, HID])
    w2b = din("w2b", [HID, HID])
    wo = din("wo", [HID, OUT])
    b1b_bc = din("b1b_bc", [128, HID])
    b2a_bc = din("b2a_bc", [128, HID])
    b2b_bc = din("b2b_bc", [128, HID])
    bo_bc = din("bo_bc", [64, OUT])
    zr1 = din("zr1", [1, HID])
    zr2 = din("zr2", [1, HID])
    zr3 = din("zr3", [1, HID])

    out_ext = nc.dram_tensor("out", [NG, OUT], f32, kind="ExternalOutput")

    AXMAX = mybir.AluOpType.max
    AXADD = mybir.AluOpType.add
    RELU = mybir.ActivationFunctionType.Relu
    COPY = mybir.ActivationFunctionType.Copy
    RG = [list(range(NC))]

    ET = S // TILE
    GB = G - GMID
    ETA = GMID // GPT        # edge tiles in half A

    with tile.TileContext(nc) as tc:
        with tc.tile_pool(name="const", bufs=1) as cp, \
             tc.tile_pool(name="dram", bufs=1, space="DRAM") as dr:
            ident = cp.tile([128, 128], f32)
            make_identity(nc, ident[:])

            G1A = dr.tile([GMID + 1, HID], f32)
            G1B = dr.tile([GB + 1, HID], f32)
            G2A = dr.tile([GMID + 1, HID], f32)
            G2B = dr.tile([GB + 1, HID], f32)
            hp_sh = [dr.tile([CHB0D, HID], f32, name="hp_shA"),
                     dr.tile([CHB1D - CHB0D, HID], f32, name="hp_shB"),
                     dr.tile([NPCX - CHB1D, HID], f32, name="hp_shC")]
            hp_full = dr.tile([NTOT, HID], f32)
            H2A = dr.tile([CHAD, HID], f32)
            H2B = dr.tile([NPCX - CHA, HID], f32)
            G3t = dr.tile([G3 + 1, HID], f32)
            pr_in = dr.tile([NG, HID], f32)
            pr_out = dr.tile([NG, HID], f32, addr_space="Shared")

            def tree_max(sp, tiles, shape, tag, bufs=2):
                cur = list(tiles)
                rnd = 0
                while len(cur) > 1:
                    nxt = []
                    for i in range(0, len(cur) - 1, 2):
                        o = sp.tile(shape, f32, tag=f"{tag}r{rnd}_{i}",
                                    name="tmx", bufs=bufs)
                        nc.vector.tensor_tensor(out=o[:], in0=cur[i][:],
                                                in1=cur[i + 1][:], op=AXMAX)
                        nxt.append(o)
                    if len(cur) % 2:
                        nxt.append(cur[-1])
                    cur = nxt
                    rnd += 1
                return cur[0]

            def level2_tree(sp, g_dram, idx_fn, nparts, kk, tag,
                            width=8, bufs=2):
                acc = None
                for k0 in range(0, kk, width):
                    gs = []
                    for k in range(k0, min(k0 + width, kk)):
                        kx = k - k0
                        it = sp.tile([nparts, 1], i32, tag=f"{tag}i{kx}",
                                     name="it", bufs=bufs)
                        nc.sync.dma_start(out=it[:], in_=idx_fn(k))
                        gt_ = sp.tile([nparts, HID], f32, tag=f"{tag}g{kx}",
                                      name="gt_", bufs=bufs)
                        nc.gpsimd.indirect_dma_start(
                            out=gt_[:], out_offset=None, in_=g_dram[:, :],
                            in_offset=bass.IndirectOffsetOnAxis(ap=it[:, :1],
                                                                axis=0))
                        gs.append(gt_)
                    cr = tree_max(sp, gs, [nparts, HID], tag, bufs=bufs)
                    if acc is None:
                        acc = cr
                    else:
                        nacc = sp.tile([nparts, HID], f32, tag=f"{tag}acc",
                                       name="nacc", bufs=2)
                        nc.vector.tensor_tensor(out=nacc[:], in0=cr[:],
                                                in1=acc[:], op=AXMAX)
                        acc = nacc
                return acc

            # ================= conv stage (edge phase + node phase) =========
            def conv_stage(conv2):
                g_a = G2A if conv2 else G1A
                g_b = G2B if conv2 else G1B
                zrow = zr2 if conv2 else zr1
                wB_dram = w2b if conv2 else w1b
                with tc.tile_pool(name="wp", bufs=1) as wp, \
                     tc.tile_pool(name="sp", bufs=2) as sp, \
                     tc.tile_pool(name="npp", bufs=2) as np_, \
                     tc.tile_pool(name="pl", bufs=2) as pl, \
                     tc.tile_pool(name="pp", bufs=2, space="PSUM") as pp:
                    wB = wp.tile([128, 8 * HID], F32R)
                    nc.sync.dma_start(
                        out=wB[:].rearrange("p (k h) -> p k h", h=HID),
                        in_=wB_dram[:, :].rearrange(
                            "(k p) h -> p k h", p=128).bitcast(F32R))
                    if not conv2:
                        wA = wp.tile([8, HID], F32R)
                        nc.sync.dma_start(out=wA[:], in_=w1a[:, :].bitcast(F32R))
                    else:
                        wA = wp.tile([4, HID], F32R)
                        nc.sync.dma_start(out=wA[:], in_=w2ar[:, :].bitcast(F32R))

                    z = wp.tile([1, HID], f32)
                    nc.sync.dma_start(out=z[:], in_=zrow[:, :])
                    nc.sync.dma_start(out=g_a[GMID:GMID + 1, :], in_=z[:])
                    nc.sync.dma_start(out=g_b[GB:GB + 1, :], in_=z[:])

                    # ---- node phase, emitted interleaved with edge tiles ----
                    if not conv2:
                        wN = wp.tile([128, 8 * HID], F32R)
                        nc.sync.dma_start(
                            out=wN[:].rearrange("p (k h) -> p k h", h=HID),
                            in_=w2an[:, :].rearrange(
                                "(k p) h -> p k h", p=128).bitcast(F32R))
                        b1 = wp.tile([128, HID], f32)
                        nc.sync.dma_start(out=b1[:], in_=b1b_bc[:, :])
                        b2 = wp.tile([128, HID], f32)
                        nc.sync.dma_start(out=b2[:], in_=b2a_bc[:, :])
                    else:
                        b1 = wp.tile([128, HID], f32)
                        nc.sync.dma_start(out=b1[:], in_=b2b_bc[:, :])
                        wN = b2 = None

                    def node_tile(nt):
                        in_a = nt * 128 < CHAD
                        g_tab = g_a if in_a else g_b
                        acc = level2_tree(
                            np_, g_tab,
                            lambda k: idx2[nt * 128:(nt + 1) * 128, k:k + 1],
                            128, K2, "l2", width=6, bufs=1)
                        hpre = np_.tile([128, HID], f32, tag="hpre")
                        nc.vector.tensor_tensor(out=hpre[:], in0=acc[:],
                                                in1=b1[:], op=AXADD)
                        rows = min(128, NPCX - nt * 128)
                        if not conv2:
                            hts = []
                            for c in range(8):
                                tps = pp.tile([128, 128], f32, space="PSUM",
                                              tag="tp")
                                nc.tensor.transpose(
                                    out=tps[:],
                                    in_=hpre[:, c * 128:(c + 1) * 128],
                                    identity=ident[:])
                                ht = np_.tile([128, 128], F32R, tag=f"ht{c}",
                                              name=f"ht{c}")
                                nc.scalar.activation(ht[:], tps[:], RELU)
                                hts.append(ht)
                            ch, ro = (0, nt * 128) if nt * 128 < CHB0D else \
                                (1, nt * 128 - CHB0D) if nt * 128 < CHB1D \
                                else (2, nt * 128 - CHB1D)
                            for hh in range(2):
                                hps = pp.tile([128, 512], f32, space="PSUM",
                                              tag="hps")
                                for c in range(8):
                                    nc.tensor.matmul(
                                        out=hps[:],
                                        lhsT=hts[c][:],
                                        rhs=wN[:, c * HID + hh * 512:
                                               c * HID + hh * 512 + 512],
                                        start=(c == 0), stop=(c == 7))
                                hpsb = np_.tile([128, 512], f32, tag="hpsb")
                                nc.vector.tensor_tensor(
                                    out=hpsb[:], in0=hps[:],
                                    in1=b2[:, hh * 512:hh * 512 + 512],
                                    op=AXADD)
                                nc.sync.dma_start(
                                    out=hp_sh[ch][ro:ro + rows,
                                                  hh * 512:hh * 512 + 512],
                                    in_=hpsb[0:rows, :])
                            end = nt * 128 + rows
                            bases = [0, NC * CHB0D, NC * CHB1D, NC * NPCX]
                            for ci, bd in enumerate((CHB0D, CHB1D, NPCX)):
                                if end == bd:
                                    nc.gpsimd.collective_compute(
                                        "AllGather", mybir.AluOpType.bypass,
                                        replica_groups=RG,
                                        ins=[hp_sh[ci][:].opt()],
                                        outs=[hp_full[bases[ci]:bases[ci + 1],
                                                      :].opt()])
                        else:
                            h2 = np_.tile([128, HID], f32, tag="h2", bufs=1)
                            nc.scalar.activation(h2[:], hpre[:], RELU)
                            ch, ro = (0, nt * 128) if in_a else \
                                (1, nt * 128 - CHAD)
                            h2_tab = H2A if in_a else H2B
                            nc.sync.dma_start(
                                out=h2_tab[ro:ro + rows, :],
                                in_=h2[0:rows, :])

                    def pool_l1_tile(pt):
                        it = pl.tile([128, 1], i32, tag="pit", bufs=2)
                        nc.sync.dma_start(
                            out=it[:],
                            in_=pslot[pt * PTILE:(pt + 1) * PTILE, None])
                        h2_tab = H2A if pt < PMID // GP_TILE else H2B
                        gat = pl.tile([128, HID], f32, tag="pgat", bufs=2)
                        nc.gpsimd.indirect_dma_start(
                            out=gat[:], out_offset=None, in_=h2_tab[:, :],
                            in_offset=bass.IndirectOffsetOnAxis(
                                ap=it[:, :1], axis=0))
                        t64 = pl.tile([64, HID], f32, tag="t64", bufs=1)
                        nc.sync.dma_start(out=t64[:], in_=gat[64:128, :])
                        m1 = pl.tile([64, HID], f32, tag="m1", bufs=1)
                        nc.vector.tensor_tensor(out=m1[:], in0=gat[0:64, :],
                                                in1=t64[:], op=AXMAX)
                        t32 = pl.tile([32, HID], f32, tag="t32", bufs=1)
                        nc.sync.dma_start(out=t32[:], in_=m1[32:64, :])
                        m2 = pl.tile([32, HID], f32, tag="m2", bufs=1)
                        nc.vector.tensor_tensor(out=m2[:], in0=m1[0:32, :],
                                                in1=t32[:], op=AXMAX)
                        t16 = pl.tile([16, HID], f32, tag="t16", bufs=1)
                        nc.sync.dma_start(out=t16[:], in_=m2[16:32, :])
                        g16 = pl.tile([16, HID], f32, tag="g16", bufs=1)
                        nc.vector.tensor_tensor(out=g16[:], in0=m2[0:16, :],
                                                in1=t16[:], op=AXMAX)
                        nc.sync.dma_start(
                            out=G3t[pt * GP_TILE:(pt + 1) * GP_TILE, :],
                            in_=g16[:])

                    nta = CHAD // 128
                    node_after = {ETA - 1: list(range(nta))}

                    # ---- edge tiles ----
                    for t in range(ET):
                        ts = slice(t * TILE, (t + 1) * TILE)
                        us = []
                        if not conv2:
                            xe = sp.tile([8, TILE], F32R, tag="xe")
                            nc.sync.dma_start(out=xe[:],
                                              in_=xeT8[:, ts].bitcast(F32R))
                            for m in range(8):
                                ups = pp.tile([128, TILE], f32, space="PSUM",
                                              tag="ups")
                                nc.tensor.matmul(
                                    out=ups[:],
                                    lhsT=wA[:, m * 128:(m + 1) * 128],
                                    rhs=xe[:], start=True, stop=True)
                                u = sp.tile([128, TILE], F32R, tag=f"u{m}",
                                            name=f"u{m}")
                                nc.scalar.activation(u[:], ups[:], RELU)
                                us.append(u)
                        else:
                            re = sp.tile([4, TILE], F32R, tag="xe")
                            nc.sync.dma_start(out=re[:],
                                              in_=relT4[:, ts].bitcast(F32R))
                            us = [sp.tile([128, TILE], F32R, tag=f"u{m}",
                                          name=f"u{m}")
                                  for m in range(8)]
                            for q in range(4):
                                base = t * TILE + q * 128
                                it = sp.tile([128, 1], i32, tag="sit", bufs=3)
                                nc.sync.dma_start(
                                    out=it[:], in_=srcg[base:base + 128, None])
                                vg = sp.tile([128, HID], f32, tag="vg", bufs=2)
                                nc.gpsimd.indirect_dma_start(
                                    out=vg[:], out_offset=None,
                                    in_=hp_full[:, :],
                                    in_offset=bass.IndirectOffsetOnAxis(
                                        ap=it[:, :1], axis=0))
                                rps = pp.tile([128, HID], f32, space="PSUM",
                                              tag="rps", bufs=1)
                                for hh in range(2):
                                    nc.tensor.matmul(
                                        out=rps[:, hh * 512:(hh + 1) * 512],
                                        lhsT=re[:, q * 128:(q + 1) * 128],
                                        rhs=wA[:, hh * 512:(hh + 1) * 512],
                                        start=True, stop=True)
                                vr = sp.tile([128, HID], f32, tag="vr")
                                nc.vector.tensor_tensor(
                                    out=vr[:], in0=vg[:], in1=rps[:], op=AXADD)
                                for c in range(8):
                                    tps = pp.tile([128, 128], f32, space="PSUM",
                                                  tag="tp")
                                    nc.tensor.transpose(
                                        out=tps[:],
                                        in_=vr[:, c * 128:(c + 1) * 128],
                                        identity=ident[:])
                                    nc.scalar.activation(
                                        us[c][:, q * 128:(q + 1) * 128],
                                        tps[:], RELU)

                        grow = sp.tile([GPT, HID], f32, tag="grow")
                        for m in range(8):
                            mps = pp.tile([128, TILE], f32, space="PSUM",
                                          tag="mps")
                            for k in range(8):
                                nc.tensor.matmul(
                                    out=mps[:],
                                    lhsT=wB[:, k * HID + m * 128:
                                            k * HID + (m + 1) * 128],
                                    rhs=us[k][:],
                                    start=(k == 0), stop=(k == 7))
                            gt = sp.tile([128, GPT], f32, tag="gt")
                            nc.vector.tensor_reduce(
                                out=gt[:],
                                in_=mps[:].rearrange("p (g k) -> p g k", k=K1),
                                axis=mybir.AxisListType.X, op=AXMAX)
                            gtp = pp.tile([GPT, 128], f32, space="PSUM",
                                          tag="tp")
                            nc.tensor.transpose(out=gtp[:], in_=gt[:],
                                                identity=ident[:])
                            nc.scalar.activation(
                                grow[:, m * 128:(m + 1) * 128], gtp[:], COPY)
                        if t < ETA:
                            nc.sync.dma_start(
                                out=g_a[t * GPT:(t + 1) * GPT, :], in_=grow[:])
                        else:
                            r0 = t * GPT - GMID
                            nc.sync.dma_start(
                                out=g_b[r0:r0 + GPT, :], in_=grow[:])
                        for nt in node_after.get(t, ()):
                            node_tile(nt)


                    for nt in range(CHAD // 128, NT):
                        node_tile(nt)
                    if conv2:
                        for pt in range(S3 // PTILE):
                            pool_l1_tile(pt)
                        z3 = wp.tile([1, HID], f32, name="z3t")
                        nc.sync.dma_start(out=z3[:], in_=zr3[:, :])
                        nc.sync.dma_start(out=G3t[G3:G3 + 1, :], in_=z3[:])

            # ================= pooling + final =================
            def pool_and_final():
                with tc.tile_pool(name="sp", bufs=2) as sp, \
                     tc.tile_pool(name="pp", bufs=2, space="PSUM") as pp:
                    pacc = level2_tree(
                        sp, G3t, lambda k: idx3[:, k:k + 1], 64, K3,
                        "l2p", width=6, bufs=2)
                    nc.sync.dma_start(out=pr_in[:], in_=pacc[:])
                    nc.gpsimd.collective_compute(
                        "AllReduce", AXMAX, replica_groups=RG,
                        ins=[pr_in[:].opt()], outs=[pr_out[:].opt()])

                    wO = sp.tile([128, 8 * OUT], F32R, tag="wo", bufs=1)
                    nc.sync.dma_start(
                        out=wO[:].rearrange("p (k h) -> p k h", h=OUT),
                        in_=wo[:, :].rearrange(
                            "(k p) h -> p k h", p=128).bitcast(F32R))
                    bO = sp.tile([64, OUT], f32, tag="bo", bufs=1)
                    nc.sync.dma_start(out=bO[:], in_=bo_bc[:, :])
                    po = sp.tile([64, HID], f32, tag="po", bufs=1)
                    nc.sync.dma_start(out=po[:], in_=pr_out[:])
                    ops = pp.tile([64, OUT], f32, space="PSUM", tag="ops")
                    for c in range(8):
                        tps = pp.tile([128, 64], f32, space="PSUM", tag="ptp")
                        nc.tensor.transpose(
                            out=tps[:], in_=po[:, c * 128:(c + 1) * 128],
                            identity=ident[0:64, 0:64])
                        ptc = sp.tile([128, 64], F32R, tag="ptc")
                        nc.scalar.activation(ptc[:], tps[:], COPY)
                        nc.tensor.matmul(
                            out=ops[:], lhsT=ptc[:],
                            rhs=wO[:, c * OUT:(c + 1) * OUT],
                            start=(c == 0), stop=(c == 7))
                    osb = sp.tile([64, OUT], f32, tag="osb")
                    nc.vector.tensor_tensor(out=osb[:], in0=ops[:], in1=bO[:],
                                            op=AXADD)
                    nc.sync.dma_start(out=out_ext[:, :], in_=osb[:])

            conv_stage(conv2=False)
            conv_stage(conv2=True)
            pool_and_final()

    nc.compile()
    return nc


# ----------------------------------------------------------------------------
# entry point
# ----------------------------------------------------------------------------

def _install_ntff_hook_shim():
    """The axon NTFF profiling glue (antenv.axon_hooks) is absent on some
    images; synthesize it from trn_agent_boot so trace=True works (and
    doesn't crash kernel() if a caller sets BASS_TRACE)."""
    import sys
    import types
    try:
        import antenv.axon_hooks  # noqa: F401
        return
    except ImportError:
        pass
    try:
        import antenv
        from trn_agent_boot.trn_boot import _ntff_profile_via_ctypes
        hook = _ntff_profile_via_ctypes("/opt/axon/libaxon_pjrt.so")
        mod = types.ModuleType("antenv.axon_hooks")
        mod.get_axon_ntff_profile_hook = lambda: hook
        mod.set_axon_ntff_profile_hook = lambda h: None
        antenv.axon_hooks = mod
        sys.modules["antenv.axon_hooks"] = mod
    except Exception:
        pass


def kernel(**inputs) -> np.ndarray:
    global LAST_RESULTS
    in_maps, sizes = _preprocess(inputs)
    if sizes not in _CACHE:
        _CACHE[sizes] = _build_program(*sizes)
    nc = _CACHE[sizes]
    trace = bool(os.environ.get("BASS_TRACE"))
    if trace:
        _install_ntff_hook_shim()
    try:
        res = run_bass_kernel_spmd(nc, in_maps, core_ids=list(range(NC)),
                                   trace=trace)
    except Exception:
        if not trace:
            raise
        os.environ["BASS_NEVER_TRACE"] = "1"
        try:
            res = run_bass_kernel_spmd(nc, in_maps,
                                       core_ids=list(range(NC)), trace=False)
        finally:
            del os.environ["BASS_NEVER_TRACE"]
    LAST_RESULTS = res
    return np.asarray(res.results[0]["out"], np.float32)


# revision 29
# speedup vs baseline: 1.0350x; 1.0350x over previous
"""Trainium2 Bass kernel for the PointNet-style GNN (nn_PointNet_36137854829226).

Self-contained: takes FULL inputs, shards internally across 8 NeuronCores,
returns the FULL [64, 512] output.

Per core (SPMD over 8 cores):
  - Edges sorted by dst; nodes sharded into 8 ranges balanced by group count
    (24 quantile cuts = 3 segments per core).
  - Per-dst edges padded into groups of K1=8 slots (pad = duplicate edge).
  - Conv math in transposed orientation M^T = [hid, edge-slots]:
      layer1:  U^T = relu(W1a_aug^T @ Xe^T)           (K=8 contraction)
      layer2:  M^T = W1b^T @ U^T  (8x8 chunked, float32r)
    Level-1 segment max = DVE tensor_reduce(max) over a [128, G, 8] PSUM
    view; results are PE-transposed on the fly into row-major G-table rows.
    Level-2 = K2 indirect-DMA gathers + DVE max tree;
    relu(max + b) with empty segments handled by a -bias zero-row.
  - All per-node tables (G tables, H2, hp AllGather chunks) are split into
    the 3 segments so each segment's node phase + collective chunk overlaps
    the remaining edge tiles of the same conv.
  - conv2 gathers hp rows (hp = h @ w2a[:1024] + b2a, AllGathered) by edge
    src via indirect DMA, adds rel @ w2a[1024:], relu, PE-transposes.
  - Pooling: two-level max over nodes grouped by graph (level-1 overlapped
    with conv2), AllReduce(max), then pooled @ wo + bo on every core.
"""
import os
import numpy as np

from concourse import bass, bacc, tile, mybir
from concourse.bass_utils import run_bass_kernel_spmd
from concourse.masks import make_identity

dt = mybir.dt

N = 10000
E = 160000
HID = 1024
OUT = 512
NG = 64
NC = 8
NSEG = 3
K1 = 8                   # edge slots per group
K1P = 8                  # pool slots per group
TILE = 512               # edge slots per device tile
GPT = TILE // K1         # groups per edge tile
PTILE = 128              # pool slots per tile
GP_TILE = PTILE // K1P   # pool groups per tile

F32R = dt.float32r

LAST_RESULTS = None      # test harness reads exec_time_ns from here
_CACHE = {}


# ----------------------------------------------------------------------------
# host preprocessing
# ----------------------------------------------------------------------------

def _ceil(a, b):
    return -(-a // b)


def _groups_for_range(counts, offs, lo, hi):
    slot_edge = []
    goff = np.zeros(hi - lo + 1, np.int64)
    for n in range(lo, hi):
        deg = int(counts[n])
        gn = _ceil(deg, K1) if deg > 0 else 0
        goff[n - lo + 1] = goff[n - lo] + gn
        o = offs[n]
        for g in range(gn):
            for k in range(K1):
                slot_edge.append(o + min(g * K1 + k, deg - 1))
    return slot_edge, goff


def _balanced_cuts(counts):
    """NC*NSEG+1 node boundaries at group-count quantiles."""
    gcnt = -(-counts // K1)
    cum = np.concatenate([[0], np.cumsum(gcnt)])
    total = cum[-1]
    cuts = [0]
    for q in range(1, NC * NSEG):
        cuts.append(int(np.searchsorted(cum, total * q / (NC * NSEG))))
    cuts.append(N)
    return cuts


def _build_edge_groups(src, dst):
    order = np.argsort(dst, kind="stable")
    s_dst = dst[order]
    counts = np.bincount(s_dst, minlength=N)
    offs = np.concatenate([[0], np.cumsum(counts)])
    cuts = _balanced_cuts(counts)

    segs = [[] for _ in range(NC)]   # per core: NSEG x (slot_edge, goff)
    g_max = [0] * NSEG
    n_max = [0] * NSEG
    k2_all = 1
    for c in range(NC):
        for s in range(NSEG):
            lo = cuts[c * NSEG + s]
            hi = cuts[c * NSEG + s + 1]
            se, goff = _groups_for_range(counts, offs, lo, hi)
            g_max[s] = max(g_max[s], int(goff[-1]))
            n_max[s] = max(n_max[s], hi - lo)
            gn = np.diff(goff)
            if len(gn) and gn.max() > 0:
                k2_all = max(k2_all, int(gn.max()))
            segs[c].append((se, goff))

    NTS = [_ceil(n_max[s], 128) for s in range(NSEG)]       # node tiles/seg
    TOFF = np.concatenate([[0], np.cumsum(NTS)])            # tile offsets
    NPCX = 128 * int(TOFF[-1])
    GS = [_ceil(g_max[s], GPT) * GPT for s in range(NSEG)]  # groups/seg
    GOFF = np.concatenate([[0], np.cumsum(GS)])
    G = int(GOFF[-1])
    S = G * K1
    K2 = k2_all

    out = []
    for c in range(NC):
        se_parts = []
        idx2 = np.empty((NPCX, K2), np.int32)
        for s in range(NSEG):
            se, goff = segs[c][s]
            se_parts.append(np.array(se, np.int64))
            se_parts.append(np.zeros(GS[s] * K1 - len(se), np.int64))
            r0 = int(TOFF[s]) * 128
            idx2[r0:int(TOFF[s + 1]) * 128] = GS[s]   # zrow of seg table
            gn = np.diff(goff)
            for n in range(len(gn)):
                if gn[n] > 0:
                    for k in range(K2):
                        idx2[r0 + n, k] = goff[n] + min(k, gn[n] - 1)
        out.append((np.concatenate(se_parts), idx2))
    return out, cuts, (S, G, tuple(GS), K2, NPCX, tuple(NTS)), order


def _pool_groups_for_range(b, lo, hi):
    slot_node = []
    gl = []
    for g in range(NG):
        nodes = np.nonzero(b[lo:hi] == g)[0]  # within-segment local ids
        if len(nodes) == 0:
            continue
        gn = _ceil(len(nodes), K1P)
        gl.append((g, gn))
        for j in range(gn):
            for k in range(K1P):
                slot_node.append(nodes[min(j * K1P + k, len(nodes) - 1)])
    return slot_node, gl


def _build_pool_groups(batch, cuts):
    per_core = []
    p_max = [0] * NSEG
    for c in range(NC):
        lo0 = cuts[c * NSEG]
        b = batch[lo0:cuts[(c + 1) * NSEG]]
        parts = []
        for s in range(NSEG):
            lo = cuts[c * NSEG + s] - lo0
            hi = cuts[c * NSEG + s + 1] - lo0
            sn, gl = _pool_groups_for_range(b, lo, hi)
            p_max[s] = max(p_max[s], sum(g for _, g in gl))
            parts.append((sn, gl))
        per_core.append(parts)

    PS = [_ceil(p_max[s], GP_TILE) * GP_TILE for s in range(NSEG)]
    POFF = np.concatenate([[0], np.cumsum(PS)])
    G3 = int(POFF[-1])
    S3 = G3 * K1P

    out = []
    k3_all = 1
    pc2 = []
    for c in range(NC):
        sn_parts = []
        gmap = {}
        for s in range(NSEG):
            sn, gl = per_core[c][s]
            sn_parts.append(np.array(sn, np.int64))
            sn_parts.append(np.zeros(PS[s] * K1P - len(sn), np.int64))
            off = int(POFF[s])
            for g, gn in gl:
                gmap.setdefault(g, []).extend(range(off, off + gn))
                off += gn
        sn_all = np.concatenate(sn_parts)
        sn_t = np.empty_like(sn_all)
        for t in range(S3 // PTILE):
            blk = sn_all[t * PTILE:(t + 1) * PTILE].reshape(GP_TILE, K1P)
            sn_t[t * PTILE:(t + 1) * PTILE] = blk.T.reshape(-1)
        k3_all = max(k3_all, max((len(v) for v in gmap.values()), default=1))
        pc2.append((sn_t, gmap))
    K3 = k3_all
    for c in range(NC):
        sn_t, gmap = pc2[c]
        idx3 = np.full((NG, K3), G3, np.int32)       # zrow of G3 table
        for g, rows in gmap.items():
            for k in range(K3):
                idx3[g, k] = rows[min(k, len(rows) - 1)]
        out.append((sn_t, idx3))
    return out, (S3, G3, tuple(PS), K3)


def _preprocess(inputs):
    x = np.ascontiguousarray(np.asarray(inputs["x"], np.float32))
    ei = np.asarray(inputs["edge_index"])
    batch = np.asarray(inputs["batch"])
    src, dst = ei[0].astype(np.int64), ei[1].astype(np.int64)

    cores, cuts, esz, order = _build_edge_groups(src, dst)
    pools, psz = _build_pool_groups(batch, cuts)
    S, G, GS, K2, NPCX, NTS = esz
    S3, G3, PS, K3 = psz
    s_src, s_dst = src[order], dst[order]

    w1a = np.asarray(inputs["w1a"], np.float32)
    b1a = np.asarray(inputs["b1a"], np.float32)
    w1b = np.asarray(inputs["w1b"], np.float32)
    b1b = np.asarray(inputs["b1b"], np.float32)
    w2a = np.asarray(inputs["w2a"], np.float32)
    b2a = np.asarray(inputs["b2a"], np.float32)
    w2b = np.asarray(inputs["w2b"], np.float32)
    b2b = np.asarray(inputs["b2b"], np.float32)
    wo = np.asarray(inputs["wo"], np.float32)
    bo = np.asarray(inputs["bo"], np.float32)

    w1a_aug = np.zeros((8, HID), np.float32)
    w1a_aug[0:6] = w1a
    w1a_aug[6] = b1a
    w2ar = np.zeros((4, HID), np.float32)
    w2ar[0:3] = w2a[HID:HID + 3]

    common = dict(
        w1a=np.ascontiguousarray(w1a_aug),
        w1b=np.ascontiguousarray(w1b),
        w2an=np.ascontiguousarray(w2a[:HID]),
        w2ar=np.ascontiguousarray(w2ar),
        w2b=np.ascontiguousarray(w2b),
        wo=np.ascontiguousarray(wo),
        b1b_bc=np.ascontiguousarray(np.broadcast_to(b1b, (128, HID))),
        b2a_bc=np.ascontiguousarray(np.broadcast_to(b2a, (128, HID))),
        b2b_bc=np.ascontiguousarray(np.broadcast_to(b2b, (128, HID))),
        bo_bc=np.ascontiguousarray(np.broadcast_to(bo, (64, OUT))),
        zr1=np.ascontiguousarray(-b1b[None, :]),
        zr2=np.ascontiguousarray(-b2b[None, :]),
        zr3=np.zeros((1, HID), np.float32),
    )

    # hp_full (AllGathered) row mapping for a global node id
    TOFF = np.concatenate([[0], np.cumsum(NTS)])
    cuts_a = np.array(cuts)
    seg_rows = np.array([128 * NTS[s] for s in range(NSEG)])
    ag_base = np.concatenate([[0], np.cumsum(NC * seg_rows)])

    in_maps = []
    for c in range(NC):
        se, idx2 = cores[c]
        sn_t, idx3 = pools[c]
        gsrc = s_src[se]
        gdst = s_dst[se]
        rel = (x[gsrc] - x[gdst]).T                       # [3, S]
        xeT8 = np.zeros((8, S), np.float32)
        xeT8[0:3] = x[gsrc].T
        xeT8[3:6] = rel
        xeT8[6] = 1.0
        relT4 = np.zeros((4, S), np.float32)
        relT4[0:3] = rel
        q = np.searchsorted(cuts_a, gsrc, side="right") - 1
        cq, sq = q // NSEG, q % NSEG
        j = gsrc - cuts_a[q]
        srcg_ag = ag_base[sq] + cq * seg_rows[sq] + j
        m = dict(common)
        m.update(
            xeT8=np.ascontiguousarray(xeT8),
            relT4=np.ascontiguousarray(relT4),
            srcg=np.ascontiguousarray(srcg_ag.astype(np.int32)),
            idx2=np.ascontiguousarray(idx2),
            pslot=np.ascontiguousarray(sn_t.astype(np.int32)),
            idx3=np.ascontiguousarray(idx3),
        )
        in_maps.append(m)
    return in_maps, (S, G, GS, K2, NPCX, NTS, S3, G3, PS, K3)


# ----------------------------------------------------------------------------
# device program
# ----------------------------------------------------------------------------

def _build_program(S, G, GS, K2, NPCX, NTS, S3, G3, PS, K3):
    nc = bacc.Bacc("TRN2", target_bir_lowering=False, debug=False,
                   num_devices=NC)
    f32 = dt.float32
    i32 = dt.int32

    def din(name, shape, dtype=f32):
        return nc.dram_tensor(name, shape, dtype, kind="ExternalInput")

    xeT8 = din("xeT8", [8, S])
    relT4 = din("relT4", [4, S])
    srcg = din("srcg", [S], i32)
    idx2 = din("idx2", [NPCX, K2], i32)
    pslot = din("pslot", [S3], i32)
    idx3 = din("idx3", [NG, K3], i32)
    w1a = din("w1a", [8, HID])
    w1b = din("w1b", [HID, HID])
    w2an = din("w2an", [HID, HID])
    w2ar = din("w2ar", [4, HID])
    w2b = din("w2b", [HID, HID])
    wo = din("wo", [HID, OUT])
    b1b_bc = din("b1b_bc", [128, HID])
    b2a_bc = din("b2a_bc", [128, HID])
    b2b_bc = din("b2b_bc", [128, HID])
    bo_bc = din("bo_bc", [64, OUT])
    zr1 = din("zr1", [1, HID])
    zr2 = din("zr2", [1, HID])
    zr3 = din("zr3", [1, HID])

    out_ext = nc.dram_tensor("out", [NG, OUT], f32, kind="ExternalOutput")

    AXMAX = mybir.AluOpType.max
    AXADD = mybir.AluOpType.add
    RELU = mybir.ActivationFunctionType.Relu
    COPY = mybir.ActivationFunctionType.Copy
    RG = [list(range(NC))]

    ET = S // TILE
    NT = NPCX // 128
    TOFF = [0]
    for s in range(NSEG):
        TOFF.append(TOFF[-1] + NTS[s])
    GOFF = [0]
    for s in range(NSEG):
        GOFF.append(GOFF[-1] + GS[s])
    TE = [GOFF[s + 1] // GPT for s in range(NSEG)]   # edge tile end per seg
    POFF = [0]
    for s in range(NSEG):
        POFF.append(POFF[-1] + PS[s])
    seg_rows = [128 * NTS[s] for s in range(NSEG)]
    ag_base = [0]
    for s in range(NSEG):
        ag_base.append(ag_base[-1] + NC * seg_rows[s])

    def seg_of_tile(nt):
        for s in range(NSEG):
            if nt < TOFF[s + 1]:
                return s
        raise AssertionError

    with tile.TileContext(nc) as tc:
        with tc.tile_pool(name="const", bufs=1) as cp, \
             tc.tile_pool(name="dram", bufs=1, space="DRAM") as dr:
            ident = cp.tile([128, 128], f32)
            make_identity(nc, ident[:])

            G1 = [dr.tile([GS[s] + 1, HID], f32, name=f"G1_{s}")
                  for s in range(NSEG)]
            G2 = [dr.tile([GS[s] + 1, HID], f32, name=f"G2_{s}")
                  for s in range(NSEG)]
            hp_sh = [dr.tile([seg_rows[s], HID], f32, name=f"hp_sh{s}")
                     for s in range(NSEG)]
            hp_full = dr.tile([ag_base[-1], HID], f32)
            H2 = [dr.tile([seg_rows[s], HID], f32, name=f"H2_{s}")
                  for s in range(NSEG)]
            G3t = dr.tile([G3 + 1, HID], f32)
            pr_in = dr.tile([NG, HID], f32)
            pr_out = dr.tile([NG, HID], f32, addr_space="Shared")

            def tree_max(sp, tiles, shape, tag, bufs=2):
                cur = list(tiles)
                rnd = 0
                while len(cur) > 1:
                    nxt = []
                    for i in range(0, len(cur) - 1, 2):
                        o = sp.tile(shape, f32, tag=f"{tag}r{rnd}_{i}",
                                    name="tmx", bufs=bufs)
                        nc.vector.tensor_tensor(out=o[:], in0=cur[i][:],
                                                in1=cur[i + 1][:], op=AXMAX)
                        nxt.append(o)
                    if len(cur) % 2:
                        nxt.append(cur[-1])
                    cur = nxt
                    rnd += 1
                return cur[0]

            def level2_tree(sp, g_dram, idx_fn, nparts, kk, tag,
                            width=6, bufs=2):
                acc = None
                for k0 in range(0, kk, width):
                    gs = []
                    for k in range(k0, min(k0 + width, kk)):
                        kx = k - k0
                        it = sp.tile([nparts, 1], i32, tag=f"{tag}i{kx}",
                                     name="it", bufs=bufs)
                        nc.sync.dma_start(out=it[:], in_=idx_fn(k))
                        gt_ = sp.tile([nparts, HID], f32, tag=f"{tag}g{kx}",
                                      name="gt_", bufs=bufs)
                        nc.gpsimd.indirect_dma_start(
                            out=gt_[:], out_offset=None, in_=g_dram[:, :],
                            in_offset=bass.IndirectOffsetOnAxis(ap=it[:, :1],
                                                                axis=0))
                        gs.append(gt_)
                    cr = tree_max(sp, gs, [nparts, HID], tag, bufs=bufs)
                    if acc is None:
                        acc = cr
                    else:
                        nacc = sp.tile([nparts, HID], f32, tag=f"{tag}acc",
                                       name="nacc", bufs=2)
                        nc.vector.tensor_tensor(out=nacc[:], in0=cr[:],
                                                in1=acc[:], op=AXMAX)
                        acc = nacc
                return acc

            # ================= conv stage (edge + node, interleaved) ========
            def conv_stage(conv2):
                g_tabs = G2 if conv2 else G1
                zrow = zr2 if conv2 else zr1
                wB_dram = w2b if conv2 else w1b
                with tc.tile_pool(name="wp", bufs=1) as wp, \
                     tc.tile_pool(name="sp", bufs=2) as sp, \
                     tc.tile_pool(name="npp", bufs=2) as np_, \
                     tc.tile_pool(name="pl", bufs=2) as pl, \
                     tc.tile_pool(name="pp", bufs=2, space="PSUM") as pp:
                    wB = wp.tile([128, 8 * HID], F32R)
                    nc.sync.dma_start(
                        out=wB[:].rearrange("p (k h) -> p k h", h=HID),
                        in_=wB_dram[:, :].rearrange(
                            "(k p) h -> p k h", p=128).bitcast(F32R))
                    if not conv2:
                        wA = wp.tile([8, HID], F32R)
                        nc.sync.dma_start(out=wA[:], in_=w1a[:, :].bitcast(F32R))
                    else:
                        wA = wp.tile([4, HID], F32R)
                        nc.sync.dma_start(out=wA[:], in_=w2ar[:, :].bitcast(F32R))

                    z = wp.tile([1, HID], f32)
                    nc.sync.dma_start(out=z[:], in_=zrow[:, :])
                    for s in range(NSEG):
                        nc.sync.dma_start(out=g_tabs[s][GS[s]:GS[s] + 1, :],
                                          in_=z[:])

                    if not conv2:
                        wN = wp.tile([128, 8 * HID], F32R)
                        nc.sync.dma_start(
                            out=wN[:].rearrange("p (k h) -> p k h", h=HID),
                            in_=w2an[:, :].rearrange(
                                "(k p) h -> p k h", p=128).bitcast(F32R))
                        b1 = wp.tile([128, HID], f32)
                        nc.sync.dma_start(out=b1[:], in_=b1b_bc[:, :])
                        b2 = wp.tile([128, HID], f32)
                        nc.sync.dma_start(out=b2[:], in_=b2a_bc[:, :])
                    else:
                        b1 = wp.tile([128, HID], f32)
                        nc.sync.dma_start(out=b1[:], in_=b2b_bc[:, :])
                        wN = b2 = None

                    def node_tile(nt):
                        sg = seg_of_tile(nt)
                        ro = (nt - TOFF[sg]) * 128
                        acc = level2_tree(
                            np_, g_tabs[sg],
                            lambda k: idx2[nt * 128:(nt + 1) * 128, k:k + 1],
                            128, K2, "l2", bufs=1)
                        hpre = np_.tile([128, HID], f32, tag="hpre")
                        nc.vector.tensor_tensor(out=hpre[:], in0=acc[:],
                                                in1=b1[:], op=AXADD)
                        if not conv2:
                            hts = []
                            for c in range(8):
                                tps = pp.tile([128, 128], f32, space="PSUM",
                                              tag="tp")
                                nc.tensor.transpose(
                                    out=tps[:],
                                    in_=hpre[:, c * 128:(c + 1) * 128],
                                    identity=ident[:])
                                ht = np_.tile([128, 128], F32R, tag=f"ht{c}",
                                              name=f"ht{c}")
                                nc.scalar.activation(ht[:], tps[:], RELU)
                                hts.append(ht)
                            for hh in range(2):
                                hps = pp.tile([128, 512], f32, space="PSUM",
                                              tag="hps")
                                for c in range(8):
                                    nc.tensor.matmul(
                                        out=hps[:],
                                        lhsT=hts[c][:],
                                        rhs=wN[:, c * HID + hh * 512:
                                               c * HID + hh * 512 + 512],
                                        start=(c == 0), stop=(c == 7))
                                hpsb = np_.tile([128, 512], f32, tag="hpsb")
                                nc.vector.tensor_tensor(
                                    out=hpsb[:], in0=hps[:],
                                    in1=b2[:, hh * 512:hh * 512 + 512],
                                    op=AXADD)
                                nc.sync.dma_start(
                                    out=hp_sh[sg][ro:ro + 128,
                                                  hh * 512:hh * 512 + 512],
                                    in_=hpsb[:, :])
                            if nt == TOFF[sg + 1] - 1:
                                nc.gpsimd.collective_compute(
                                    "AllGather", mybir.AluOpType.bypass,
                                    replica_groups=RG,
                                    ins=[hp_sh[sg][:].opt()],
                                    outs=[hp_full[ag_base[sg]:ag_base[sg + 1],
                                                  :].opt()])
                        else:
                            h2 = np_.tile([128, HID], f32, tag="h2", bufs=1)
                            nc.scalar.activation(h2[:], hpre[:], RELU)
                            nc.sync.dma_start(
                                out=H2[sg][ro:ro + 128, :], in_=h2[:, :])

                    def pool_l1_tile(pt):
                        for s in range(NSEG):
                            if pt * GP_TILE < POFF[s + 1]:
                                sg = s
                                break
                        it = pl.tile([128, 1], i32, tag="pit", bufs=2)
                        nc.sync.dma_start(
                            out=it[:],
                            in_=pslot[pt * PTILE:(pt + 1) * PTILE, None])
                        gat = pl.tile([128, HID], f32, tag="pgat", bufs=2)
                        nc.gpsimd.indirect_dma_start(
                            out=gat[:], out_offset=None, in_=H2[sg][:, :],
                            in_offset=bass.IndirectOffsetOnAxis(
                                ap=it[:, :1], axis=0))
                        t64 = pl.tile([64, HID], f32, tag="t64", bufs=1)
                        nc.sync.dma_start(out=t64[:], in_=gat[64:128, :])
                        m1 = pl.tile([64, HID], f32, tag="m1", bufs=1)
                        nc.vector.tensor_tensor(out=m1[:], in0=gat[0:64, :],
                                                in1=t64[:], op=AXMAX)
                        t32 = pl.tile([32, HID], f32, tag="t32", bufs=1)
                        nc.sync.dma_start(out=t32[:], in_=m1[32:64, :])
                        m2 = pl.tile([32, HID], f32, tag="m2", bufs=1)
                        nc.vector.tensor_tensor(out=m2[:], in0=m1[0:32, :],
                                                in1=t32[:], op=AXMAX)
                        t16 = pl.tile([16, HID], f32, tag="t16", bufs=1)
                        nc.sync.dma_start(out=t16[:], in_=m2[16:32, :])
                        g16 = pl.tile([16, HID], f32, tag="g16", bufs=1)
                        nc.vector.tensor_tensor(out=g16[:], in0=m2[0:16, :],
                                                in1=t16[:], op=AXMAX)
                        nc.sync.dma_start(
                            out=G3t[pt * GP_TILE:(pt + 1) * GP_TILE, :],
                            in_=g16[:])

                    def emit_seg_nodes(sg):
                        for nt in range(TOFF[sg], TOFF[sg + 1]):
                            node_tile(nt)
                        if conv2:
                            for pt in range(POFF[sg] // GP_TILE,
                                            POFF[sg + 1] // GP_TILE):
                                pool_l1_tile(pt)

                    node_after = {TE[s] - 1: s for s in range(NSEG - 1)}

                    # ---- edge tiles (node segments interleaved) ----
                    for t in range(ET):
                        ts = slice(t * TILE, (t + 1) * TILE)
                        us = []
                        if not conv2:
                            xe = sp.tile([8, TILE], F32R, tag="xe")
                            nc.sync.dma_start(out=xe[:],
                                              in_=xeT8[:, ts].bitcast(F32R))
                            for m in range(8):
                                ups = pp.tile([128, TILE], f32, space="PSUM",
                                              tag="ups")
                                nc.tensor.matmul(
                                    out=ups[:],
                                    lhsT=wA[:, m * 128:(m + 1) * 128],
                                    rhs=xe[:], start=True, stop=True)
                                u = sp.tile([128, TILE], F32R, tag=f"u{m}",
                                            name=f"u{m}")
                                nc.scalar.activation(u[:], ups[:], RELU)
                                us.append(u)
                        else:
                            re = sp.tile([4, TILE], F32R, tag="xe")
                            nc.sync.dma_start(out=re[:],
                                              in_=relT4[:, ts].bitcast(F32R))
                            us = [sp.tile([128, TILE], F32R, tag=f"u{m}",
                                          name=f"u{m}")
                                  for m in range(8)]
                            for q in range(4):
                                base = t * TILE + q * 128
                                it = sp.tile([128, 1], i32, tag="sit", bufs=3)
                                nc.sync.dma_start(
                                    out=it[:], in_=srcg[base:base + 128, None])
                                vg = sp.tile([128, HID], f32, tag="vg", bufs=2)
                                nc.gpsimd.indirect_dma_start(
                                    out=vg[:], out_offset=None,
                                    in_=hp_full[:, :],
                                    in_offset=bass.IndirectOffsetOnAxis(
                                        ap=it[:, :1], axis=0))
                                rps = pp.tile([128, HID], f32, space="PSUM",
                                              tag="rps", bufs=1)
                                for hh in range(2):
                                    nc.tensor.matmul(
                                        out=rps[:, hh * 512:(hh + 1) * 512],
                                        lhsT=re[:, q * 128:(q + 1) * 128],
                                        rhs=wA[:, hh * 512:(hh + 1) * 512],
                                        start=True, stop=True)
                                vr = sp.tile([128, HID], f32, tag="vr")
                                nc.vector.tensor_tensor(
                                    out=vr[:], in0=vg[:], in1=rps[:], op=AXADD)
                                for c in range(8):
                                    tps = pp.tile([128, 128], f32, space="PSUM",
                                                  tag="tp")
                                    nc.tensor.transpose(
                                        out=tps[:],
                                        in_=vr[:, c * 128:(c + 1) * 128],
                                        identity=ident[:])
                                    nc.scalar.activation(
                                        us[c][:, q * 128:(q + 1) * 128],
                                        tps[:], RELU)

                        grow = sp.tile([GPT, HID], f32, tag="grow")
                        for m in range(8):
                            mps = pp.tile([128, TILE], f32, space="PSUM",
                                          tag="mps")
                            for k in range(8):
                                nc.tensor.matmul(
                                    out=mps[:],
                                    lhsT=wB[:, k * HID + m * 128:
                                            k * HID + (m + 1) * 128],
                                    rhs=us[k][:],
                                    start=(k == 0), stop=(k == 7))
                            gt = sp.tile([128, GPT], f32, tag="gt")
                            nc.vector.tensor_reduce(
                                out=gt[:],
                                in_=mps[:].rearrange("p (g k) -> p g k", k=K1),
                                axis=mybir.AxisListType.X, op=AXMAX)
                            gtp = pp.tile([GPT, 128], f32, space="PSUM",
                                          tag="tp")
                            nc.tensor.transpose(out=gtp[:], in_=gt[:],
                                                identity=ident[:])
                            nc.scalar.activation(
                                grow[:, m * 128:(m + 1) * 128], gtp[:], COPY)
                        for s in range(NSEG):
                            if t < TE[s]:
                                r0 = t * GPT - GOFF[s]
                                nc.sync.dma_start(
                                    out=g_tabs[s][r0:r0 + GPT, :],
                                    in_=grow[:])
                                break
                        if t in node_after:
                            emit_seg_nodes(node_after[t])

                    emit_seg_nodes(NSEG - 1)
                    if conv2:
                        z3 = wp.tile([1, HID], f32, name="z3t")
                        nc.sync.dma_start(out=z3[:], in_=zr3[:, :])
                        nc.sync.dma_start(out=G3t[G3:G3 + 1, :], in_=z3[:])

            # ================= pool level-2 + final =================
            def pool_and_final():
                with tc.tile_pool(name="sp", bufs=2) as sp, \
                     tc.tile_pool(name="pp", bufs=2, space="PSUM") as pp:
                    pacc = level2_tree(
                        sp, G3t, lambda k: idx3[:, k:k + 1], 64, K3,
                        "l2p", width=6, bufs=2)
                    nc.sync.dma_start(out=pr_in[:], in_=pacc[:])
                    nc.gpsimd.collective_compute(
                        "AllReduce", AXMAX, replica_groups=RG,
                        ins=[pr_in[:].opt()], outs=[pr_out[:].opt()])

                    wO = sp.tile([128, 8 * OUT], F32R, tag="wo", bufs=1)
                    nc.sync.dma_start(
                        out=wO[:].rearrange("p (k h) -> p k h", h=OUT),
                        in_=wo[:, :].rearrange(
                            "(k p) h -> p k h", p=128).bitcast(F32R))
                    bO = sp.tile([64, OUT], f32, tag="bo", bufs=1)
                    nc.sync.dma_start(out=bO[:], in_=bo_bc[:, :])
                    po = sp.tile([64, HID], f32, tag="po", bufs=1)
                    nc.sync.dma_start(out=po[:], in_=pr_out[:])
                    ops = pp.tile([64, OUT], f32, space="PSUM", tag="ops")
                    for c in range(8):
                        tps = pp.tile([128, 64], f32, space="PSUM", tag="ptp")
                        nc.tensor.transpose(
                            out=tps[:], in_=po[:, c * 128:(c + 1) * 128],
                            identity=ident[0:64, 0:64])
                        ptc = sp.tile([128, 64], F32R, tag="ptc")
                        nc.scalar.activation(ptc[:], tps[:], COPY)
                        nc.tensor.matmul(
                            out=ops[:], lhsT=ptc[:],
                            rhs=wO[:, c * OUT:(c + 1) * OUT],
                            start=(c == 0), stop=(c == 7))
                    osb = sp.tile([64, OUT], f32, tag="osb")
                    nc.vector.tensor_tensor(out=osb[:], in0=ops[:], in1=bO[:],
                                            op=AXADD)
                    nc.sync.dma_start(out=out_ext[:, :], in_=osb[:])

            conv_stage(conv2=False)
            conv_stage(conv2=True)
            pool_and_final()

    nc.compile()
    return nc


# ----------------------------------------------------------------------------
# entry point
# ----------------------------------------------------------------------------

def _install_ntff_hook_shim():
    """The axon NTFF profiling glue (antenv.axon_hooks) is absent on some
    images; synthesize it from trn_agent_boot so trace=True works (and
    doesn't crash kernel() if a caller sets BASS_TRACE)."""
    import sys
    import types
    try:
        import antenv.axon_hooks  # noqa: F401
        return
    except ImportError:
        pass
    try:
        import antenv
        from trn_agent_boot.trn_boot import _ntff_profile_via_ctypes
        hook = _ntff_profile_via_ctypes("/opt/axon/libaxon_pjrt.so")
        mod = types.ModuleType("antenv.axon_hooks")
        mod.get_axon_ntff_profile_hook = lambda: hook
        mod.set_axon_ntff_profile_hook = lambda h: None
        antenv.axon_hooks = mod
        sys.modules["antenv.axon_hooks"] = mod
    except Exception:
        pass


def kernel(**inputs) -> np.ndarray:
    global LAST_RESULTS
    in_maps, sizes = _preprocess(inputs)
    if sizes not in _CACHE:
        _CACHE[sizes] = _build_program(*sizes)
    nc = _CACHE[sizes]
    trace = bool(os.environ.get("BASS_TRACE"))
    if trace:
        _install_ntff_hook_shim()
    try:
        res = run_bass_kernel_spmd(nc, in_maps, core_ids=list(range(NC)),
                                   trace=trace)
    except Exception:
        if not trace:
            raise
        os.environ["BASS_NEVER_TRACE"] = "1"
        try:
            res = run_bass_kernel_spmd(nc, in_maps,
                                       core_ids=list(range(NC)), trace=False)
        finally:
            del os.environ["BASS_NEVER_TRACE"]
    LAST_RESULTS = res
    return np.asarray(res.results[0]["out"], np.float32)
